# revision 10
# baseline (speedup 1.0000x reference)
"""AttnBlock v5: fp8 DoubleRow attention core + head/tail scheduling.

Sharding: core = (batch b in {0,1}) x (query slice s in {0..3}, 1024
queries).  Each core redundantly computes full V^T for its batch
(avoids cross-core collectives), attention for its query slice only.
The host rolls x columns per core so the core's query block is always
columns 0:1024 -- identical program, per-core data.

Math: h = GN(x) = A_c * x + B_c per channel (A, B from runtime stats).
  q = (wq*A)@x + (wq@B + bq)     weight columns scaled on device
  k = (wk*A)@x   (k-bias dropped: per-query-constant under softmax)
  v = (wv*A)@x + const; v-bias folded into the projection bias:
      bp_dev = bp + wp@bv + wp@(wv@B).

v5 over v4 (224us -> 158us -> this):
  - DMA descriptor issue was ~41us serialized on the Sync engine; now
    weights/consts/x8 issue from the (otherwise idle) GpSimd DGE ring,
    Sync carries only x chunks + output, and consts/weights are batched
    into two tensors (29 descriptors total instead of 63).
  - GroupNorm stats split DVE (6 of 8 groups, bn_stats) / ACT (2 of 8
    groups, Identity+Square accum_out passes): stats wall ~16us -> ~12.
  - finalize in 256-column chunks so output DMA starts earlier and the
    last-chunk tail is a quarter the size.
"""

import os
import sys

import numpy as np

for _p in ("/opt/trn_rl_repo", "/root/.axon_site/_ro/trn_rl_repo"):
    if os.path.isdir(_p) and _p not in sys.path:
        sys.path.insert(0, _p)

B, C, H, W = 2, 512, 64, 64
N = H * W
G = 32
GS = C // G
EPS = 1e-6
NCORES = 8
QS = N // 4               # 1024 queries per core
NHALF = 2                 # key halves
JQ = N // NHALF           # 2048 keys per half
JT = JQ // 128            # 16 key tiles per half
KT2 = JT // 2             # 8 DoubleRow key groups per half
ICH = 512                 # query chunk
NCH = QS // ICH           # 2 chunks
CT = C // 128             # 4 channel tiles
NG = 2                    # DoubleRow channel groups (256 ch each)
FCH = 256                 # finalize chunk
SCALE = float(C) ** -0.5
WVS = 16.0                # wv prescale into fp8
ESHIFT = -3.0             # exp(s + ESHIFT): keep e in fp8e4m3 range
                          # (max scaled score ~7.2; e4m3 max normal 240);
                          # a constant shift cancels in the softmax ratio
NDVE = 6                  # of 8 per-tile stat groups on DVE (rest ACT)

_CACHE = {}


def _build():
    import contextlib

    import concourse.mybir as mybir
    import concourse.tile as tile
    from concourse import bacc
    from concourse.alu_op_type import AluOpType as alu

    f32 = mybir.dt.float32
    bf16 = mybir.dt.bfloat16
    f8 = mybir.dt.float8e4
    AF = mybir.ActivationFunctionType
    PM = mybir.MatmulPerfMode

    nc = bacc.Bacc("TRN2", target_bir_lowering=False, debug=False,
                   num_devices=NCORES)

    xbf = nc.dram_tensor("xbf", [C, N], bf16, kind="ExternalInput").ap()
    x8d = [nc.dram_tensor(f"x8_{g}", [128, 2, N], f8,
                          kind="ExternalInput").ap() for g in range(NG)]
    # wts = [m0T | wvT | wpT/16] side by side; consts4 = [gamma | beta |
    # bp_eff | qkbc] as columns
    wts = nc.dram_tensor("wts", [C, 3 * C], bf16, kind="ExternalInput").ap()
    consts4 = nc.dram_tensor("consts4", [C, 4], f32,
                             kind="ExternalInput").ap()
    sel = nc.dram_tensor("sel", [128, 8], f32, kind="ExternalInput").ap()
    selT = nc.dram_tensor("selT", [8, 128], f32, kind="ExternalInput").ap()
    ones8 = nc.dram_tensor("ones8", [128, 2, 128], f8,
                           kind="ExternalInput").ap()
    out_d = nc.dram_tensor("out", [C, QS], f32, kind="ExternalOutput").ap()

    def mm(ps, lhsT, rhs, start, stop):
        nc.tensor.matmul(ps, lhsT, rhs, start=start, stop=stop)

    def mm8(ps, lhsT, rhs, start, stop):
        nc.tensor.matmul(ps, lhsT, rhs, start=start, stop=stop,
                         perf_mode=PM.DoubleRow)

    with tile.TileContext(nc) as tc:
        outer = contextlib.ExitStack()
        with outer:
            cpool = outer.enter_context(tc.tile_pool(name="const", bufs=1))
            x_p = outer.enter_context(tc.tile_pool(name="xbf", bufs=1))
            x8_p = outer.enter_context(tc.tile_pool(name="x8", bufs=1))
            acc_p = outer.enter_context(tc.tile_pool(name="acc", bufs=1))
            w_p = outer.enter_context(tc.tile_pool(name="wts", bufs=1))
            q8_p = outer.enter_context(tc.tile_pool(name="q8", bufs=1))
            v8_p = outer.enter_context(tc.tile_pool(name="v8", bufs=KT2))
            e8_p = outer.enter_context(tc.tile_pool(name="e8", bufs=KT2 + 2))
            f_p = outer.enter_context(tc.tile_pool(name="fin", bufs=1))

            # ---- x chunks on the Sync DGE ring (stats critical path) ----
            x_t = []
            for t in range(CT):
                row = []
                for c in range(N // 1024):
                    xt = x_p.tile([128, 1024], bf16, tag=f"x{t}_{c}",
                                  name=f"x{t}_{c}")
                    nc.sync.dma_start(
                        xt[:], xbf[t * 128:(t + 1) * 128,
                                   c * 1024:(c + 1) * 1024])
                    row.append(xt)
                x_t.append(row)

            def xsl(ci, start, size):
                c, off = divmod(start, 1024)
                assert off + size <= 1024
                return x_t[ci][c][:, off:off + size]

            # ---- everything else on the GpSimd DGE ring ----
            c4_t = []
            for t in range(CT):
                c4 = cpool.tile([128, 4], f32, tag=f"c4_{t}")
                nc.gpsimd.dma_start(c4[:], consts4[t * 128:(t + 1) * 128, :])
                c4_t.append(c4)
            gam_t = [c4_t[t][:, 0:1] for t in range(CT)]
            bet_t = [c4_t[t][:, 1:2] for t in range(CT)]
            bp_t = [c4_t[t][:, 2:3] for t in range(CT)]
            qkbc_t = [c4_t[t][:, 3:4] for t in range(CT)]
            sel_t = cpool.tile([128, 8], f32, tag="sel")
            nc.gpsimd.dma_start(sel_t[:], sel[:])
            selT_t = cpool.tile([8, 128], f32, tag="selT")
            nc.gpsimd.dma_start(selT_t[:], selT[:])
            ones8_t = cpool.tile([128, 2, 128], f8, tag="ones8")
            nc.gpsimd.dma_start(ones8_t[:], ones8[:])
            esh_t = cpool.tile([128, 1], f32, tag="esh")
            nc.vector.memset(esh_t[:], ESHIFT)
            wts_t = []
            for t in range(CT):
                wt = w_p.tile([128, 3 * C], bf16, tag=f"wts{t}")
                nc.gpsimd.dma_start(wt[:], wts[t * 128:(t + 1) * 128, :])
                wts_t.append(wt)
            m0_t = [wts_t[t][:, 0:C] for t in range(CT)]
            wv_t = [wts_t[t][:, C:2 * C] for t in range(CT)]
            wp_t = [wts_t[t][:, 2 * C:3 * C] for t in range(CT)]
            x8_t = []
            for g in range(NG):
                xt8 = x8_p.tile([128, 2, N], f8, tag=f"x8_{g}",
                                name=f"x8_{g}")
                nc.gpsimd.dma_start(xt8[:], x8d[g][:])
                x8_t.append(xt8)

            den_acc = acc_p.tile([1, QS], f32, tag="den")
            recip = acc_p.tile([1, QS], f32, tag="recip")
            acc_t = [acc_p.tile([128, QS], f32, tag=f"acc{t}", name=f"acc{t}")
                     for t in range(CT)]

            # ---- GroupNorm stats: DVE bn_stats (groups 0..5) in parallel
            # ---- with ACT Identity/Square accum passes (groups 6, 7)
            with tc.tile_pool(name="small", bufs=1) as sm_p, \
                 tc.tile_pool(name="scr", bufs=2) as scr_p, \
                 tc.tile_pool(name="stat_ps", bufs=1, space="PSUM") as stat_ps, \
                 tc.tile_pool(name="ab_ps", bufs=2, space="PSUM") as ab_ps, \
                 tc.tile_pool(name="b_ps", bufs=2, space="PSUM") as b_ps:
                ps_st = stat_ps.tile([8, 8], f32, tag="st")
                for t in range(CT):
                    st = sm_p.tile([128, NDVE, 6], f32, tag=f"bnst{t}")
                    for g in range(NDVE):
                        nc.vector.bn_stats(st[:, g, :],
                                           xsl(t, g * 512, 512))
                    ag = sm_p.tile([128, 2], f32, tag=f"bnag{t}")
                    nc.vector.bn_aggr(ag[:], st[:])
                    sx = sm_p.tile([128, 2], f32, tag=f"sx{t}")
                    sq = sm_p.tile([128, 2], f32, tag=f"sq{t}")
                    for k in range(8 - NDVE):
                        g = NDVE + k
                        scr = scr_p.tile([128, 512], bf16, tag="scr")
                        nc.scalar.activation(scr[:], xsl(t, g * 512, 512),
                                             AF.Identity,
                                             accum_out=sx[:, k:k + 1])
                        scr2 = scr_p.tile([128, 512], bf16, tag="scr")
                        nc.scalar.activation(scr2[:], xsl(t, g * 512, 512),
                                             AF.Square,
                                             accum_out=sq[:, k:k + 1])
                    # combine: mean over 4096 and E[x^2] over 4096
                    frac = NDVE / 8.0
                    u = sm_p.tile([128, 1], f32, tag=f"u{t}")
                    nc.vector.tensor_tensor(u[:], sx[:, 0:1], sx[:, 1:2],
                                            alu.add)
                    mean_t = sm_p.tile([128, 1], f32, tag=f"mean{t}")
                    nc.vector.tensor_scalar(mean_t[:], ag[:, 0:1], frac,
                                            None, op0=alu.mult)
                    nc.vector.scalar_tensor_tensor(
                        mean_t[:], u[:], 1.0 / N, mean_t[:],
                        op0=alu.mult, op1=alu.add)
                    v = sm_p.tile([128, 1], f32, tag=f"v{t}")
                    nc.vector.tensor_tensor(v[:], sq[:, 0:1], sq[:, 1:2],
                                            alu.add)
                    s2_t = sm_p.tile([128, 1], f32, tag=f"s2{t}")
                    nc.vector.tensor_tensor(s2_t[:], ag[:, 0:1], ag[:, 0:1],
                                            alu.mult)
                    nc.vector.tensor_tensor(s2_t[:], s2_t[:], ag[:, 1:2],
                                            alu.add)
                    nc.vector.tensor_scalar(s2_t[:], s2_t[:], frac,
                                            None, op0=alu.mult)
                    nc.vector.scalar_tensor_tensor(
                        s2_t[:], v[:], 1.0 / N, s2_t[:],
                        op0=alu.mult, op1=alu.add)
                    nc.tensor.matmul(ps_st[:, t:t + 1], sel_t[:], mean_t[:],
                                     start=True, stop=True)
                    nc.tensor.matmul(ps_st[:, 4 + t:5 + t], sel_t[:],
                                     s2_t[:], start=True, stop=True)
                st_sb = sm_p.tile([8, 8], f32, tag="st_sb")
                nc.vector.tensor_copy(st_sb[:], ps_st[:])
                mean = sm_p.tile([8, 4], f32, tag="mean")
                nc.vector.tensor_scalar(mean[:], st_sb[:, 0:4],
                                        1.0 / GS, None, op0=alu.mult)
                msq = sm_p.tile([8, 4], f32, tag="msq")
                nc.vector.tensor_scalar(msq[:], st_sb[:, 4:8],
                                        1.0 / GS, None, op0=alu.mult)
                var = sm_p.tile([8, 4], f32, tag="var")
                nc.vector.tensor_tensor(var[:], mean[:], mean[:], alu.mult)
                nc.vector.tensor_tensor(var[:], msq[:], var[:], alu.subtract)
                nc.vector.tensor_scalar(var[:], var[:], EPS, None, op0=alu.add)
                sd = sm_p.tile([8, 4], f32, tag="sd")
                nc.scalar.activation(sd[:], var[:], AF.Sqrt)
                rstd = sm_p.tile([8, 4], f32, tag="rstd")
                nc.vector.reciprocal(rstd[:], sd[:])
                A_t, A16_t, Bb_t = [], [], []
                for t in range(CT):
                    ps_ab = ab_ps.tile([128, 2], f32, tag="ab")
                    nc.tensor.matmul(ps_ab[:, 0:1], selT_t[:],
                                     rstd[:, t:t + 1], start=True, stop=True)
                    nc.tensor.matmul(ps_ab[:, 1:2], selT_t[:],
                                     mean[:, t:t + 1], start=True, stop=True)
                    ab = cpool.tile([128, 2], f32, tag=f"ab{t}")
                    nc.vector.tensor_copy(ab[:], ps_ab[:])
                    At = cpool.tile([128, 1], f32, tag=f"A{t}")
                    nc.vector.tensor_tensor(At[:], ab[:, 0:1], gam_t[t],
                                            alu.mult)
                    At16 = cpool.tile([128, 1], f32, tag=f"A16_{t}")
                    nc.vector.tensor_scalar(At16[:], At[:], WVS, None,
                                            op0=alu.mult)
                    Bt = cpool.tile([128, 1], f32, tag=f"B{t}")
                    nc.vector.tensor_tensor(Bt[:], ab[:, 1:2], At[:], alu.mult)
                    nc.vector.tensor_tensor(Bt[:], bet_t[t], Bt[:],
                                            alu.subtract)
                    Bb = cpool.tile([128, 1], bf16, tag=f"Bb{t}")
                    nc.vector.tensor_copy(Bb[:], Bt[:])
                    A_t.append(At)
                    A16_t.append(At16)
                    Bb_t.append(Bb)

                # wv8[g][:, i, :] = fp8(A16 * wv_raw rows), t = 2g + i
                # (emitted before the q path so half-0 v-production can
                # start as soon as x8 lands)
                wv8_t = []
                for g in range(NG):
                    w8 = w_p.tile([128, 2, C], f8, tag=f"wv8_{g}")
                    for i in range(2):
                        t = 2 * g + i
                        nc.scalar.activation(w8[:, i, :], wv_t[t],
                                             AF.Identity, scale=A16_t[t][:])
                    wv8_t.append(w8)

                # bias terms from RAW weights:
                #   qkb = M0@B + wk^T bq (host const);  Abias = A*qkb
                #   tv  = wv@B  (for the projection-bias fold)
                abias_t, tvb_t = [], []
                for co in range(CT):
                    ps_b = b_ps.tile([128, 2], f32, tag="bb")
                    for ci in range(CT):
                        mm(ps_b[:, 0:1],
                           m0_t[ci][:, co * 128:(co + 1) * 128], Bb_t[ci][:],
                           ci == 0, ci == CT - 1)
                    for ci in range(CT):
                        mm(ps_b[:, 1:2],
                           wv_t[ci][:, co * 128:(co + 1) * 128], Bb_t[ci][:],
                           ci == 0, ci == CT - 1)
                    ab2 = cpool.tile([128, 1], f32, tag=f"abias{co}")
                    nc.vector.tensor_tensor(ab2[:], ps_b[:, 0:1],
                                            qkbc_t[co], alu.add)
                    nc.vector.tensor_tensor(ab2[:], ab2[:], A_t[co][:],
                                            alu.mult)
                    abias_t.append(ab2)
                    tvb = cpool.tile([128, 1], bf16, tag=f"tvb{co}")
                    nc.vector.tensor_copy(tvb[:], ps_b[:, 1:2])
                    tvb_t.append(tvb)

                # scale m0 rows by A_cin in place (bf16, q-projection)
                for ci in range(CT):
                    nc.vector.tensor_scalar(m0_t[ci], m0_t[ci],
                                            A_t[ci][:], None, op0=alu.mult)

            # ---- qk projection -> fp8 DR tiles: q8[g][:, i, :] ----
            with tc.tile_pool(name="q_ps", bufs=2, space="PSUM") as q_ps:
                q8_t = [q8_p.tile([128, 2, QS], f8, tag=f"q8_{g}",
                                  name=f"q8_{g}") for g in range(NG)]
                for co in range(CT):
                    g, i = divmod(co, 2)
                    for nn in range(QS // 512):
                        ps = q_ps.tile([128, 512], f32, tag="qp")
                        for ci in range(CT):
                            mm(ps[:], m0_t[ci][:, co * 128:(co + 1) * 128],
                               xsl(ci, nn * 512, 512),
                               ci == 0, ci == CT - 1)
                        nc.scalar.activation(
                            q8_t[g][:, i, nn * 512:(nn + 1) * 512],
                            ps[:], AF.Identity,
                            bias=abias_t[co][:], scale=A_t[co][:])

            # ---- device projection bias: bpd = 16*(wp/16)@tv + bp_eff ----
            with tc.tile_pool(name="u_ps", bufs=2, space="PSUM") as u_ps:
                bpd_t = []
                for co in range(CT):
                    ps_u = u_ps.tile([128, 1], f32, tag="u")
                    for ci in range(CT):
                        mm(ps_u[:], wp_t[ci][:, co * 128:(co + 1) * 128],
                           tvb_t[ci][:], ci == 0, ci == CT - 1)
                    bpd = f_p.tile([128, 1], f32, tag=f"bpd{co}")
                    nc.vector.scalar_tensor_tensor(
                        bpd[:], ps_u[:], WVS, bp_t[co],
                        op0=alu.mult, op1=alu.add)
                    bpd_t.append(bpd)

            # ---- attention over key halves (fp8 DoubleRow) ----
            with tc.tile_pool(name="mm_ps", bufs=3, space="PSUM") as mm_ps, \
                 tc.tile_pool(name="att_ps", bufs=3, space="PSUM") as att_ps, \
                 tc.tile_pool(name="den_ps", bufs=1, space="PSUM") as den_ps:
                for half in range(NHALF):
                    j0 = half * JQ
                    v8_t = []
                    for jt in range(JT):
                        ps = mm_ps.tile([128, 512], f32, tag="mm")
                        for g in range(NG):
                            mm8(ps[:],
                                x8_t[g][:, :, j0 + jt * 128:
                                        j0 + (jt + 1) * 128],
                                wv8_t[g][:], g == 0, g == NG - 1)
                        kt2, slot = divmod(jt, 2)
                        if slot == 0:
                            vt = v8_p.tile([128, 2, C], f8, tag="v8")
                            v8_t.append(vt)
                        if jt % 4 < 2:
                            nc.scalar.copy(v8_t[kt2][:, slot, :], ps[:])
                        else:
                            nc.vector.tensor_copy(v8_t[kt2][:, slot, :],
                                                  ps[:])

                    for ch in range(NCH):
                        i0 = ch * ICH
                        e8_t = []
                        for jt in range(JT):
                            ps = mm_ps.tile([128, ICH], f32, tag="mm")
                            for g in range(NG):
                                mm8(ps[:],
                                    x8_t[g][:, :, j0 + jt * 128:
                                            j0 + (jt + 1) * 128],
                                    q8_t[g][:, :, i0:i0 + ICH],
                                    g == 0, g == NG - 1)
                            kt2, slot = divmod(jt, 2)
                            if slot == 0:
                                et = e8_p.tile([128, 2, ICH], f8, tag="e8")
                                e8_t.append(et)
                            nc.scalar.activation(e8_t[kt2][:, slot, :],
                                                 ps[:], AF.Exp, scale=SCALE,
                                                 bias=esh_t[:])
                        # denominator: all-ones stationary (every output
                        # partition carries the same key-sum; row 0 used)
                        ps_d = den_ps.tile([128, ICH], f32, tag="den")
                        for kt2 in range(KT2):
                            mm8(ps_d[:], ones8_t[:], e8_t[kt2][:],
                                kt2 == 0, kt2 == KT2 - 1)
                        if half == 0:
                            nc.vector.tensor_copy(den_acc[:, i0:i0 + ICH],
                                                  ps_d[0:1, :])
                        else:
                            nc.vector.tensor_tensor(den_acc[:, i0:i0 + ICH],
                                                    den_acc[:, i0:i0 + ICH],
                                                    ps_d[0:1, :], alu.add)
                            nc.vector.reciprocal(recip[:, i0:i0 + ICH],
                                                 den_acc[:, i0:i0 + ICH])
                        for co in range(CT):
                            ps_a = att_ps.tile([128, ICH], f32, tag="att")
                            for kt2 in range(KT2):
                                mm8(ps_a[:],
                                    v8_t[kt2][:, :, co * 128:(co + 1) * 128],
                                    e8_t[kt2][:], kt2 == 0, kt2 == KT2 - 1)
                            if half == 0:
                                nc.vector.tensor_copy(
                                    acc_t[co][:, i0:i0 + ICH], ps_a[:])
                            else:
                                nc.vector.tensor_tensor(
                                    acc_t[co][:, i0:i0 + ICH],
                                    acc_t[co][:, i0:i0 + ICH], ps_a[:],
                                    alu.add)

            # ---- finalize per 256-col chunk (overlaps attention tail) ----
            with tc.tile_pool(name="outp", bufs=4) as o_p, \
                 tc.tile_pool(name="f_ps", bufs=2, space="PSUM") as f_ps:
                rb = f_p.tile([128, QS], f32, tag="rb")
                accn_t = [f_p.tile([128, QS], bf16, tag=f"accn{t}",
                                   name=f"accn{t}") for t in range(CT)]
                for nn in range(QS // FCH):
                    sl = slice(nn * FCH, (nn + 1) * FCH)
                    nc.gpsimd.partition_broadcast(rb[:, sl], recip[:, sl])
                    for t in range(CT):
                        nc.vector.tensor_tensor(accn_t[t][:, sl],
                                                acc_t[t][:, sl], rb[:, sl],
                                                alu.mult)
                    for co in range(CT):
                        ps = f_ps.tile([128, FCH], f32, tag="f")
                        for ci in range(CT):
                            mm(ps[:], wp_t[ci][:, co * 128:(co + 1) * 128],
                               accn_t[ci][:, sl], ci == 0, ci == CT - 1)
                        ot = o_p.tile([128, FCH], f32, tag="o")
                        nc.vector.scalar_tensor_tensor(
                            ot[:], ps[:], bpd_t[co][:],
                            x_t[co][0][:, sl],
                            op0=alu.add, op1=alu.add)
                        nc.sync.dma_start(
                            out_d[co * 128:(co + 1) * 128, sl], ot[:])

    nc.compile()
    return nc


def kernel(x, gn_gamma, gn_beta, wq, bq, wk, bk, wv, bv, wp, bp):
    import ml_dtypes
    from concourse import bass_utils

    if "nc" not in _CACHE:
        _CACHE["nc"] = _build()
    nc = _CACHE["nc"]

    x = np.asarray(x, np.float32)
    f = np.float32
    bf = ml_dtypes.bfloat16
    f8 = ml_dtypes.float8_e4m3
    wq32 = np.asarray(wq, f)
    wk32 = np.asarray(wk, f)
    m0T = (wq32.T @ wk32).astype(bf)  # (wk^T wq)^T
    qkbc = (wk32.T @ np.asarray(bq, f)).reshape(C, 1)
    wvT = np.asarray(wv, f).T.astype(bf)
    wpT = (np.asarray(wp, f).T / WVS).astype(bf)
    wts = np.ascontiguousarray(np.concatenate([m0T, wvT, wpT], axis=1))
    bp_eff = (np.asarray(bp, f)
              + np.asarray(wp, f) @ np.asarray(bv, f)).reshape(C, 1)
    consts4 = np.ascontiguousarray(np.concatenate([
        np.asarray(gn_gamma, f).reshape(C, 1),
        np.asarray(gn_beta, f).reshape(C, 1),
        bp_eff, qkbc], axis=1))
    sel = np.zeros((128, 8), f)
    for p in range(128):
        sel[p, p // GS] = 1.0
    common = {
        "wts": wts, "consts4": consts4,
        "sel": sel, "selT": np.ascontiguousarray(sel.T),
        "ones8": np.ones((128, 2, 128), f8),
    }
    in_maps = []
    for core in range(NCORES):
        b, s = divmod(core, 4)
        xb = x[b].reshape(C, N)
        # roll so this core's query block occupies columns 0:QS; key order
        # is permuted identically for all key-side tensors, and softmax
        # sums are order-invariant, so the program is core-independent.
        xperm = np.ascontiguousarray(np.roll(xb, -s * QS, axis=1))
        im = {**common, "xbf": xperm.astype(bf)}
        for g in range(NG):
            x8g = xperm[g * 256:(g + 1) * 256].reshape(2, 128, N)
            im[f"x8_{g}"] = np.ascontiguousarray(
                x8g.transpose(1, 0, 2)).astype(f8)
        in_maps.append(im)

    res = bass_utils.run_bass_kernel_spmd(nc, in_maps,
                                          core_ids=list(range(NCORES)))
    _CACHE["last_result"] = res

    out = np.empty((B, C, N), np.float32)
    for core in range(NCORES):
        b, s = divmod(core, 4)
        out[b][:, s * QS:(s + 1) * QS] = res.results[core]["out"]
    return out.reshape(B, C, H, W)


# revision 13
# speedup vs baseline: 1.0798x; 1.0798x over previous
"""AttnBlock v6: all matmul stages fp8 DoubleRow; ordered DMA rings.

Sharding: core = (batch b in {0,1}) x (query slice s in {0..3}, 1024
queries).  Each core redundantly computes full V^T for its batch
(avoids cross-core collectives), attention for its query slice only.
The host rolls x columns per core so the core's query block is always
columns 0:1024 -- identical program, per-core data.

Math: h = GN(x) = A_c * x + B_c per channel (A, B from runtime stats).
  q = (wq*A)@x + (wq@B + bq)     weight columns scaled on device
  k = (wk*A)@x   (k-bias dropped: per-query-constant under softmax)
  v = (wv*A)@x + const; v-bias folded into the projection bias:
      bp_dev = bp + wp@bv + wp@(wv@B).

v6 over v5 (224 -> 158 -> 166(regr) -> this):
  - q-projection and final projection also run fp8 DoubleRow: m0 and wp
    quantized (prescaled x16 against fp8 subnormals; scale unwound in
    the q activation and the final residual add), attn output
    normalized straight into fp8 DR tiles.
  - DMA transfer ORDER restored: everything bandwidth-heavy rides the
    Sync ring strictly after the x chunks (v5 put weights on a parallel
    ring, which halved the effective x bandwidth and starved the
    GroupNorm stats); only the tiny consts ride the GpSimd ring.
  - stats split rebalanced 7 DVE / 1 ACT group (each ACT accum pass
    costs ~1us: ACTIVATE 705ns + READ_ACCUMULATOR 279ns).
  - att_ps back to 2 bufs, finalize back to 512-col chunks (v5's 3
    bufs + finer chunks pushed PSUM over 8 banks and serialized the
    attention->finalize boundary).
"""

import os
import sys

import numpy as np

for _p in ("/opt/trn_rl_repo", "/root/.axon_site/_ro/trn_rl_repo"):
    if os.path.isdir(_p) and _p not in sys.path:
        sys.path.insert(0, _p)

B, C, H, W = 2, 512, 64, 64
N = H * W
G = 32
GS = C // G
EPS = 1e-6
NCORES = 8
QS = N // 4               # 1024 queries per core
NHALF = 2                 # key halves
JQ = N // NHALF           # 2048 keys per half
JT = JQ // 128            # 16 key tiles per half
KT2 = JT // 2             # 8 DoubleRow key groups per half
ICH = 512                 # query chunk
NCH = QS // ICH           # 2 chunks
CT = C // 128             # 4 channel tiles
NG = 2                    # DoubleRow channel groups (256 ch each)
SCALE = float(C) ** -0.5
WVS = 16.0                # wv / m0 / wp prescale into fp8
ESHIFT = -3.0             # exp(s + ESHIFT): keep e in fp8e4m3 range
                          # (max scaled score ~7.2; e4m3 max normal 240);
                          # a constant shift cancels in the softmax ratio
NDVE = 7                  # of 8 per-tile stat groups on DVE (rest ACT)

_CACHE = {}


def _build():
    import contextlib

    import concourse.mybir as mybir
    import concourse.tile as tile
    from concourse import bacc
    from concourse.alu_op_type import AluOpType as alu

    f32 = mybir.dt.float32
    bf16 = mybir.dt.bfloat16
    f8 = mybir.dt.float8e4
    AF = mybir.ActivationFunctionType
    PM = mybir.MatmulPerfMode

    nc = bacc.Bacc("TRN2", target_bir_lowering=False, debug=False,
                   num_devices=NCORES)

    xbf = nc.dram_tensor("xbf", [C, N], bf16, kind="ExternalInput").ap()
    x8d = [nc.dram_tensor(f"x8_{g}", [128, 2, N], f8,
                          kind="ExternalInput").ap() for g in range(NG)]
    # wts2 = [m0T | wvT]; wpd = wp.T/16; consts4 = [gamma | beta |
    # bp_eff | qkbc] as columns
    wts2 = nc.dram_tensor("wts2", [C, 2 * C], bf16, kind="ExternalInput").ap()
    wpd = nc.dram_tensor("wpd", [C, C], bf16, kind="ExternalInput").ap()
    consts4 = nc.dram_tensor("consts4", [C, 4], f32,
                             kind="ExternalInput").ap()
    sel = nc.dram_tensor("sel", [128, 8], f32, kind="ExternalInput").ap()
    selT = nc.dram_tensor("selT", [8, 128], f32, kind="ExternalInput").ap()
    ones8 = nc.dram_tensor("ones8", [128, 2, 128], f8,
                           kind="ExternalInput").ap()
    out_d = nc.dram_tensor("out", [C, QS], f32, kind="ExternalOutput").ap()

    def mm(ps, lhsT, rhs, start, stop):
        nc.tensor.matmul(ps, lhsT, rhs, start=start, stop=stop)

    def mm8(ps, lhsT, rhs, start, stop):
        nc.tensor.matmul(ps, lhsT, rhs, start=start, stop=stop,
                         perf_mode=PM.DoubleRow)

    with tile.TileContext(nc) as tc:
        outer = contextlib.ExitStack()
        with outer:
            cpool = outer.enter_context(tc.tile_pool(name="const", bufs=1))
            x_p = outer.enter_context(tc.tile_pool(name="xbf", bufs=1))
            x8_p = outer.enter_context(tc.tile_pool(name="x8", bufs=1))
            acc_p = outer.enter_context(tc.tile_pool(name="acc", bufs=1))
            w_p = outer.enter_context(tc.tile_pool(name="wts", bufs=1))
            q8_p = outer.enter_context(tc.tile_pool(name="q8", bufs=1))
            v8_p = outer.enter_context(tc.tile_pool(name="v8", bufs=KT2))
            e8_p = outer.enter_context(tc.tile_pool(name="e8", bufs=KT2 + 2))
            f_p = outer.enter_context(tc.tile_pool(name="fin", bufs=1))

            # ---- Sync ring, in transfer-priority order: x chunks (stats
            # ---- critical), then x8, then m0/wv, then wp ----
            x_t = []
            for t in range(CT):
                row = []
                for c in range(N // 1024):
                    xt = x_p.tile([128, 1024], bf16, tag=f"x{t}_{c}",
                                  name=f"x{t}_{c}")
                    nc.sync.dma_start(
                        xt[:], xbf[t * 128:(t + 1) * 128,
                                   c * 1024:(c + 1) * 1024])
                    row.append(xt)
                x_t.append(row)

            def xsl(ci, start, size):
                c, off = divmod(start, 1024)
                assert off + size <= 1024
                return x_t[ci][c][:, off:off + size]

            x8_t = []
            for g in range(NG):
                xt8 = x8_p.tile([128, 2, N], f8, tag=f"x8_{g}",
                                name=f"x8_{g}")
                nc.sync.dma_start(xt8[:], x8d[g][:])
                x8_t.append(xt8)
            wts_t = []
            for t in range(CT):
                wt = w_p.tile([128, 2 * C], bf16, tag=f"wts{t}")
                nc.sync.dma_start(wt[:], wts2[t * 128:(t + 1) * 128, :])
                wts_t.append(wt)
            m0_t = [wts_t[t][:, 0:C] for t in range(CT)]
            wv_t = [wts_t[t][:, C:2 * C] for t in range(CT)]
            wp_t = []
            for t in range(CT):
                wt = w_p.tile([128, C], bf16, tag=f"wp{t}")
                nc.sync.dma_start(wt[:], wpd[t * 128:(t + 1) * 128, :])
                wp_t.append(wt)

            # ---- tiny consts on the GpSimd ring (no bandwidth impact) ----
            c4_t = []
            for t in range(CT):
                c4 = cpool.tile([128, 4], f32, tag=f"c4_{t}")
                nc.gpsimd.dma_start(c4[:], consts4[t * 128:(t + 1) * 128, :])
                c4_t.append(c4)
            gam_t = [c4_t[t][:, 0:1] for t in range(CT)]
            bet_t = [c4_t[t][:, 1:2] for t in range(CT)]
            bp_t = [c4_t[t][:, 2:3] for t in range(CT)]
            qkbc_t = [c4_t[t][:, 3:4] for t in range(CT)]
            sel_t = cpool.tile([128, 8], f32, tag="sel")
            nc.gpsimd.dma_start(sel_t[:], sel[:])
            selT_t = cpool.tile([8, 128], f32, tag="selT")
            nc.gpsimd.dma_start(selT_t[:], selT[:])
            ones8_t = cpool.tile([128, 2, 128], f8, tag="ones8")
            nc.gpsimd.dma_start(ones8_t[:], ones8[:])
            esh_t = cpool.tile([128, 1], f32, tag="esh")
            nc.vector.memset(esh_t[:], ESHIFT)

            den_acc = acc_p.tile([1, QS], f32, tag="den")
            recip = acc_p.tile([1, QS], f32, tag="recip")
            acc_t = [acc_p.tile([128, QS], f32, tag=f"acc{t}", name=f"acc{t}")
                     for t in range(CT)]

            # ---- GroupNorm stats: DVE bn_stats (groups 0..NDVE-1) in
            # ---- parallel with ACT Identity/Square accum (the rest)
            with tc.tile_pool(name="small", bufs=1) as sm_p, \
                 tc.tile_pool(name="scr", bufs=2) as scr_p, \
                 tc.tile_pool(name="stat_ps", bufs=1, space="PSUM") as stat_ps, \
                 tc.tile_pool(name="ab_ps", bufs=2, space="PSUM") as ab_ps, \
                 tc.tile_pool(name="b_ps", bufs=2, space="PSUM") as b_ps:
                ps_st = stat_ps.tile([8, 8], f32, tag="st")
                for t in range(CT):
                    st = sm_p.tile([128, NDVE, 6], f32, tag=f"bnst{t}")
                    for g in range(NDVE):
                        nc.vector.bn_stats(st[:, g, :],
                                           xsl(t, g * 512, 512))
                    ag = sm_p.tile([128, 2], f32, tag=f"bnag{t}")
                    nc.vector.bn_aggr(ag[:], st[:])
                    nact = 8 - NDVE
                    sx = sm_p.tile([128, nact], f32, tag=f"sx{t}")
                    sq = sm_p.tile([128, nact], f32, tag=f"sq{t}")
                    for k in range(nact):
                        g = NDVE + k
                        scr = scr_p.tile([128, 512], bf16, tag="scr")
                        nc.scalar.activation(scr[:], xsl(t, g * 512, 512),
                                             AF.Identity,
                                             accum_out=sx[:, k:k + 1])
                        scr2 = scr_p.tile([128, 512], bf16, tag="scr")
                        nc.scalar.activation(scr2[:], xsl(t, g * 512, 512),
                                             AF.Square,
                                             accum_out=sq[:, k:k + 1])
                    # combine into mean over 4096 and E[x^2] over 4096
                    frac = NDVE / 8.0
                    u = sm_p.tile([128, 1], f32, tag=f"u{t}")
                    if nact > 1:
                        nc.vector.tensor_tensor(u[:], sx[:, 0:1], sx[:, 1:2],
                                                alu.add)
                        for k in range(2, nact):
                            nc.vector.tensor_tensor(u[:], u[:],
                                                    sx[:, k:k + 1], alu.add)
                    else:
                        nc.vector.tensor_copy(u[:], sx[:])
                    mean_t = sm_p.tile([128, 1], f32, tag=f"mean{t}")
                    nc.vector.tensor_scalar(mean_t[:], ag[:, 0:1], frac,
                                            None, op0=alu.mult)
                    nc.vector.scalar_tensor_tensor(
                        mean_t[:], u[:], 1.0 / N, mean_t[:],
                        op0=alu.mult, op1=alu.add)
                    v = sm_p.tile([128, 1], f32, tag=f"v{t}")
                    if nact > 1:
                        nc.vector.tensor_tensor(v[:], sq[:, 0:1], sq[:, 1:2],
                                                alu.add)
                        for k in range(2, nact):
                            nc.vector.tensor_tensor(v[:], v[:],
                                                    sq[:, k:k + 1], alu.add)
                    else:
                        nc.vector.tensor_copy(v[:], sq[:])
                    s2_t = sm_p.tile([128, 1], f32, tag=f"s2{t}")
                    nc.vector.tensor_tensor(s2_t[:], ag[:, 0:1], ag[:, 0:1],
                                            alu.mult)
                    nc.vector.tensor_tensor(s2_t[:], s2_t[:], ag[:, 1:2],
                                            alu.add)
                    nc.vector.tensor_scalar(s2_t[:], s2_t[:], frac,
                                            None, op0=alu.mult)
                    nc.vector.scalar_tensor_tensor(
                        s2_t[:], v[:], 1.0 / N, s2_t[:],
                        op0=alu.mult, op1=alu.add)
                    nc.tensor.matmul(ps_st[:, t:t + 1], sel_t[:], mean_t[:],
                                     start=True, stop=True)
                    nc.tensor.matmul(ps_st[:, 4 + t:5 + t], sel_t[:],
                                     s2_t[:], start=True, stop=True)
                st_sb = sm_p.tile([8, 8], f32, tag="st_sb")
                nc.vector.tensor_copy(st_sb[:], ps_st[:])
                mean = sm_p.tile([8, 4], f32, tag="mean")
                nc.vector.tensor_scalar(mean[:], st_sb[:, 0:4],
                                        1.0 / GS, None, op0=alu.mult)
                msq = sm_p.tile([8, 4], f32, tag="msq")
                nc.vector.tensor_scalar(msq[:], st_sb[:, 4:8],
                                        1.0 / GS, None, op0=alu.mult)
                var = sm_p.tile([8, 4], f32, tag="var")
                nc.vector.tensor_tensor(var[:], mean[:], mean[:], alu.mult)
                nc.vector.tensor_tensor(var[:], msq[:], var[:], alu.subtract)
                nc.vector.tensor_scalar(var[:], var[:], EPS, None, op0=alu.add)
                sd = sm_p.tile([8, 4], f32, tag="sd")
                nc.scalar.activation(sd[:], var[:], AF.Sqrt)
                rstd = sm_p.tile([8, 4], f32, tag="rstd")
                nc.vector.reciprocal(rstd[:], sd[:])
                A_t, A16_t, Ai16_t, Bb_t = [], [], [], []
                for t in range(CT):
                    ps_ab = ab_ps.tile([128, 2], f32, tag="ab")
                    nc.tensor.matmul(ps_ab[:, 0:1], selT_t[:],
                                     rstd[:, t:t + 1], start=True, stop=True)
                    nc.tensor.matmul(ps_ab[:, 1:2], selT_t[:],
                                     mean[:, t:t + 1], start=True, stop=True)
                    ab = cpool.tile([128, 2], f32, tag=f"ab{t}")
                    nc.vector.tensor_copy(ab[:], ps_ab[:])
                    At = cpool.tile([128, 1], f32, tag=f"A{t}")
                    nc.vector.tensor_tensor(At[:], ab[:, 0:1], gam_t[t],
                                            alu.mult)
                    At16 = cpool.tile([128, 1], f32, tag=f"A16_{t}")
                    nc.vector.tensor_scalar(At16[:], At[:], WVS, None,
                                            op0=alu.mult)
                    Ai16 = cpool.tile([128, 1], f32, tag=f"Ai16_{t}")
                    nc.vector.tensor_scalar(Ai16[:], At[:], 1.0 / WVS, None,
                                            op0=alu.mult)
                    Bt = cpool.tile([128, 1], f32, tag=f"B{t}")
                    nc.vector.tensor_tensor(Bt[:], ab[:, 1:2], At[:], alu.mult)
                    nc.vector.tensor_tensor(Bt[:], bet_t[t], Bt[:],
                                            alu.subtract)
                    Bb = cpool.tile([128, 1], bf16, tag=f"Bb{t}")
                    nc.vector.tensor_copy(Bb[:], Bt[:])
                    A_t.append(At)
                    A16_t.append(At16)
                    Ai16_t.append(Ai16)
                    Bb_t.append(Bb)

                # fp8 DR weight tiles (prescaled x16; t = 2g + i):
                #   wv8 = fp8(A16*wv rows), m08 = fp8(A16*m0 rows),
                #   wp8 = fp8(16 * wp.T) [host sent wp.T/16 -> scale 256]
                wv8_t, m08_t, wp8_t = [], [], []
                for g in range(NG):
                    w8 = w_p.tile([128, 2, C], f8, tag=f"wv8_{g}")
                    m8 = w_p.tile([128, 2, C], f8, tag=f"m08_{g}")
                    p8 = w_p.tile([128, 2, C], f8, tag=f"wp8_{g}")
                    for i in range(2):
                        t = 2 * g + i
                        nc.scalar.activation(w8[:, i, :], wv_t[t],
                                             AF.Identity, scale=A16_t[t][:])
                        nc.scalar.activation(m8[:, i, :], m0_t[t],
                                             AF.Identity, scale=A16_t[t][:])
                        nc.scalar.activation(p8[:, i, :], wp_t[t][:],
                                             AF.Identity, scale=WVS * WVS)
                    wv8_t.append(w8)
                    m08_t.append(m8)
                    wp8_t.append(p8)

                # bias terms from RAW weights:
                #   qkb = M0@B + wk^T bq (host const);  Abias = A*qkb
                #   tv  = wv@B  (for the projection-bias fold)
                abias_t, tvb_t = [], []
                for co in range(CT):
                    ps_b = b_ps.tile([128, 2], f32, tag="bb")
                    for ci in range(CT):
                        mm(ps_b[:, 0:1],
                           m0_t[ci][:, co * 128:(co + 1) * 128], Bb_t[ci][:],
                           ci == 0, ci == CT - 1)
                    for ci in range(CT):
                        mm(ps_b[:, 1:2],
                           wv_t[ci][:, co * 128:(co + 1) * 128], Bb_t[ci][:],
                           ci == 0, ci == CT - 1)
                    ab2 = cpool.tile([128, 1], f32, tag=f"abias{co}")
                    nc.vector.tensor_tensor(ab2[:], ps_b[:, 0:1],
                                            qkbc_t[co], alu.add)
                    nc.vector.tensor_tensor(ab2[:], ab2[:], A_t[co][:],
                                            alu.mult)
                    abias_t.append(ab2)
                    tvb = cpool.tile([128, 1], bf16, tag=f"tvb{co}")
                    nc.vector.tensor_copy(tvb[:], ps_b[:, 1:2])
                    tvb_t.append(tvb)

            # ---- qk projection (fp8 DR) -> fp8 DR tiles q8[g][:, i, :]
            # psum carries 16*q (m08 prescale); activation scale A/16
            with tc.tile_pool(name="q_ps", bufs=2, space="PSUM") as q_ps:
                q8_t = [q8_p.tile([128, 2, QS], f8, tag=f"q8_{g}",
                                  name=f"q8_{g}") for g in range(NG)]
                for co in range(CT):
                    g, i = divmod(co, 2)
                    for nn in range(QS // 512):
                        ps = q_ps.tile([128, 512], f32, tag="qp")
                        for gi in range(NG):
                            mm8(ps[:],
                                m08_t[gi][:, :, co * 128:(co + 1) * 128],
                                x8_t[gi][:, :, nn * 512:(nn + 1) * 512],
                                gi == 0, gi == NG - 1)
                        nc.scalar.activation(
                            q8_t[g][:, i, nn * 512:(nn + 1) * 512],
                            ps[:], AF.Identity,
                            bias=abias_t[co][:], scale=Ai16_t[co][:])

            # ---- device projection bias: bpd = 16*(wp/16)@tv + bp_eff ----
            with tc.tile_pool(name="u_ps", bufs=2, space="PSUM") as u_ps:
                bpd_t = []
                for co in range(CT):
                    ps_u = u_ps.tile([128, 1], f32, tag="u")
                    for ci in range(CT):
                        mm(ps_u[:], wp_t[ci][:, co * 128:(co + 1) * 128],
                           tvb_t[ci][:], ci == 0, ci == CT - 1)
                    bpd = f_p.tile([128, 1], f32, tag=f"bpd{co}")
                    nc.vector.scalar_tensor_tensor(
                        bpd[:], ps_u[:], WVS, bp_t[co],
                        op0=alu.mult, op1=alu.add)
                    bpd_t.append(bpd)
                # xb = x_residual + bpd (per output-channel tile): lets the
                # finalize use one STT: out = ps/256 + xb
                xb_t = []
                for co in range(CT):
                    xb = f_p.tile([128, QS], bf16, tag=f"xb{co}")
                    nc.vector.tensor_scalar(xb[:], x_t[co][0][:],
                                            bpd_t[co][:], None, op0=alu.add)
                    xb_t.append(xb)

            # ---- attention over key halves (fp8 DoubleRow) ----
            with tc.tile_pool(name="mm_ps", bufs=3, space="PSUM") as mm_ps, \
                 tc.tile_pool(name="att_ps", bufs=2, space="PSUM") as att_ps, \
                 tc.tile_pool(name="den_ps", bufs=1, space="PSUM") as den_ps:
                for half in range(NHALF):
                    j0 = half * JQ
                    v8_t = []
                    for jt in range(JT):
                        ps = mm_ps.tile([128, 512], f32, tag="mm")
                        for g in range(NG):
                            mm8(ps[:],
                                x8_t[g][:, :, j0 + jt * 128:
                                        j0 + (jt + 1) * 128],
                                wv8_t[g][:], g == 0, g == NG - 1)
                        kt2, slot = divmod(jt, 2)
                        if slot == 0:
                            vt = v8_p.tile([128, 2, C], f8, tag="v8")
                            v8_t.append(vt)
                        if jt % 4 < 2:
                            nc.scalar.copy(v8_t[kt2][:, slot, :], ps[:])
                        else:
                            nc.vector.tensor_copy(v8_t[kt2][:, slot, :],
                                                  ps[:])

                    for ch in range(NCH):
                        i0 = ch * ICH
                        e8_t = []
                        for jt in range(JT):
                            ps = mm_ps.tile([128, ICH], f32, tag="mm")
                            for g in range(NG):
                                mm8(ps[:],
                                    x8_t[g][:, :, j0 + jt * 128:
                                            j0 + (jt + 1) * 128],
                                    q8_t[g][:, :, i0:i0 + ICH],
                                    g == 0, g == NG - 1)
                            kt2, slot = divmod(jt, 2)
                            if slot == 0:
                                et = e8_p.tile([128, 2, ICH], f8, tag="e8")
                                e8_t.append(et)
                            nc.scalar.activation(e8_t[kt2][:, slot, :],
                                                 ps[:], AF.Exp, scale=SCALE,
                                                 bias=esh_t[:])
                        # denominator: all-ones stationary (every output
                        # partition carries the same key-sum; row 0 used)
                        ps_d = den_ps.tile([128, ICH], f32, tag="den")
                        for kt2 in range(KT2):
                            mm8(ps_d[:], ones8_t[:], e8_t[kt2][:],
                                kt2 == 0, kt2 == KT2 - 1)
                        if half == 0:
                            nc.vector.tensor_copy(den_acc[:, i0:i0 + ICH],
                                                  ps_d[0:1, :])
                        else:
                            nc.vector.tensor_tensor(den_acc[:, i0:i0 + ICH],
                                                    den_acc[:, i0:i0 + ICH],
                                                    ps_d[0:1, :], alu.add)
                            nc.vector.reciprocal(recip[:, i0:i0 + ICH],
                                                 den_acc[:, i0:i0 + ICH])
                        for co in range(CT):
                            ps_a = att_ps.tile([128, ICH], f32, tag="att")
                            for kt2 in range(KT2):
                                mm8(ps_a[:],
                                    v8_t[kt2][:, :, co * 128:(co + 1) * 128],
                                    e8_t[kt2][:], kt2 == 0, kt2 == KT2 - 1)
                            if half == 0:
                                nc.vector.tensor_copy(
                                    acc_t[co][:, i0:i0 + ICH], ps_a[:])
                            else:
                                nc.vector.tensor_tensor(
                                    acc_t[co][:, i0:i0 + ICH],
                                    acc_t[co][:, i0:i0 + ICH], ps_a[:],
                                    alu.add)

            # ---- finalize per query chunk (overlaps attention tail):
            # accn8 = fp8(16*attnout), proj = DR(wp8, accn8) = 256*out,
            # out = ps/256 + (x + bpd)
            with tc.tile_pool(name="outp", bufs=3) as o_p, \
                 tc.tile_pool(name="f_ps", bufs=2, space="PSUM") as f_ps:
                rb = f_p.tile([128, QS], f32, tag="rb")
                accn8_t = [f_p.tile([128, 2, QS], f8, tag=f"accn8_{g}",
                                    name=f"accn8_{g}") for g in range(NG)]
                for nn in range(QS // 512):
                    sl = slice(nn * 512, (nn + 1) * 512)
                    nc.gpsimd.partition_broadcast(rb[:, sl], recip[:, sl])
                    for t in range(CT):
                        g, i = divmod(t, 2)
                        nc.vector.tensor_tensor(accn8_t[g][:, i, sl],
                                                acc_t[t][:, sl], rb[:, sl],
                                                alu.mult)
                    for co in range(CT):
                        ps = f_ps.tile([128, 512], f32, tag="f")
                        for g in range(NG):
                            mm8(ps[:],
                                wp8_t[g][:, :, co * 128:(co + 1) * 128],
                                accn8_t[g][:, :, sl], g == 0, g == NG - 1)
                        ot = o_p.tile([128, 512], f32, tag="o")
                        nc.vector.scalar_tensor_tensor(
                            ot[:], ps[:], 1.0 / (WVS * WVS), xb_t[co][:, sl],
                            op0=alu.mult, op1=alu.add)
                        nc.sync.dma_start(
                            out_d[co * 128:(co + 1) * 128, sl], ot[:])

    nc.compile()
    return nc


def kernel(x, gn_gamma, gn_beta, wq, bq, wk, bk, wv, bv, wp, bp):
    import ml_dtypes
    from concourse import bass_utils

    if "nc" not in _CACHE:
        _CACHE["nc"] = _build()
    nc = _CACHE["nc"]

    x = np.asarray(x, np.float32)
    f = np.float32
    bf = ml_dtypes.bfloat16
    f8 = ml_dtypes.float8_e4m3
    wq32 = np.asarray(wq, f)
    wk32 = np.asarray(wk, f)
    m0T = (wq32.T @ wk32).astype(bf)  # (wk^T wq)^T
    qkbc = (wk32.T @ np.asarray(bq, f)).reshape(C, 1)
    wvT = np.asarray(wv, f).T.astype(bf)
    wts2 = np.ascontiguousarray(np.concatenate([m0T, wvT], axis=1))
    wpd = np.ascontiguousarray((np.asarray(wp, f).T / WVS).astype(bf))
    bp_eff = (np.asarray(bp, f)
              + np.asarray(wp, f) @ np.asarray(bv, f)).reshape(C, 1)
    consts4 = np.ascontiguousarray(np.concatenate([
        np.asarray(gn_gamma, f).reshape(C, 1),
        np.asarray(gn_beta, f).reshape(C, 1),
        bp_eff, qkbc], axis=1))
    sel = np.zeros((128, 8), f)
    for p in range(128):
        sel[p, p // GS] = 1.0
    common = {
        "wts2": wts2, "wpd": wpd, "consts4": consts4,
        "sel": sel, "selT": np.ascontiguousarray(sel.T),
        "ones8": np.ones((128, 2, 128), f8),
    }
    in_maps = []
    for core in range(NCORES):
        b, s = divmod(core, 4)
        xb = x[b].reshape(C, N)
        # roll so this core's query block occupies columns 0:QS; key order
        # is permuted identically for all key-side tensors, and softmax
        # sums are order-invariant, so the program is core-independent.
        xperm = np.ascontiguousarray(np.roll(xb, -s * QS, axis=1))
        im = {**common, "xbf": xperm.astype(bf)}
        for g in range(NG):
            x8g = xperm[g * 256:(g + 1) * 256].reshape(2, 128, N)
            im[f"x8_{g}"] = np.ascontiguousarray(
                x8g.transpose(1, 0, 2)).astype(f8)
        in_maps.append(im)

    res = bass_utils.run_bass_kernel_spmd(nc, in_maps,
                                          core_ids=list(range(NCORES)))
    _CACHE["last_result"] = res

    out = np.empty((B, C, N), np.float32)
    for core in range(NCORES):
        b, s = divmod(core, 4)
        out[b][:, s * QS:(s + 1) * QS] = res.results[core]["out"]
    return out.reshape(B, C, H, W)
